# revision 6
# baseline (speedup 1.0000x reference)
"""Trainium2 Bass kernel for nn_DNM_Conv_fold (LayerNorm + M parallel 1x1 convs
+ relu(y-q) summed over M), v2: fp16 end-to-end.

Math (same restructure as v1, validated):
  - gamma folds into W host-side; W rows centered => LayerNorm mean-subtraction
    implicit in the matmul.
  - sv = sqrt(var+eps) rides the matmul as contraction row 65 with weight row
    bq = beta@W_eff - q, so psum = y - q*sv and the drain is a PURE relu
    (batchable across tiles; no per-tile scalars).
  - out = a * sum_m relu(y - q*sv), a = 1/sv applied after the m-sum via a
    transposed per-tile scale table aT [128pp, 64tile].

Layout per core: x [64, 73728] fp16 + device-computed sv row -> xt [65, 8192]
per chunk. Main MM per 128-px tile: lhsT = xt[:, 128t:128t+128] (stationary),
rhs = wca [65, 256] (moving), psum [128px, 256mo]; 4 tiles per [128, 1024]
psum span; relu-drain spans on ACT or DVE (ratio tunable); m-sum via batched
fp16 DVE adds; scale via aT broadcast; output [128, 4096] fp16 pixel-major,
host unscrambles + upcasts.

Stats: 16 x 512-px slices/chunk; x-pass (rows 0-63) and sq-pass (rows 64-127
of a separate sq tile) matmuls write (mu, e2) rows into 32-aligned psum
col-groups, 4 slices/bank => 4 ACT bank-copies [98, 512] per chunk; DMA
gathers to [64, 128] row form; var/sqrt there; sv DMA'd into xt row 64; DVE
32x32 block-transposes give svT -> aT.

Sharding: 8 cores; core k = batch k//2, pixel half k%2 (73728 px each).
"""

import sys

sys.path.insert(0, "/opt/trn_rl_repo")

import numpy as np

# ---- problem constants (hardcoded; kernel.py must be self-contained) ----
B, C, O, M, H, Wd = 4, 64, 64, 4, 384, 384
EPS = 1e-5
MO = M * O  # 256
NCORES = 8
PIX_PER_CORE = B * H * Wd // NCORES  # 73728
CHUNK = 8192
NCHUNK = PIX_PER_CORE // CHUNK  # 9
NTILE = CHUNK // 128  # 64 px-tiles per chunk
NSPAN = NTILE // 4  # 16 drain spans (4 tiles each)
NSLICE = CHUNK // 512  # 16 stat slices
# drain-span engine schedule: True -> ACT, False -> DVE
ACT_SPANS = 12  # of 16

_cache = {}


def _build(pix_per_core=PIX_PER_CORE, chunk=CHUNK, repeat=1):
    import contextlib

    from concourse import bacc, bass, tile

    mybir = bass.mybir
    f32 = mybir.dt.float32
    f16 = mybir.dt.float16
    AF = mybir.ActivationFunctionType
    ALU = mybir.AluOpType

    nchunk = pix_per_core // chunk
    ntile = chunk // 128
    nspan = ntile // 4
    nslice = chunk // 512

    nc = bacc.Bacc(None, target_bir_lowering=False)
    xin = nc.declare_dram_parameter("xin", [C, pix_per_core], f16, isOutput=False)
    wca_d = nc.declare_dram_parameter("wca", [C + 1, MO], f16, isOutput=False)
    cst_d = nc.declare_dram_parameter("cst", [128, 8], f16, isOutput=False)
    out_d = nc.declare_dram_parameter(
        "out", [128, pix_per_core // 128 * O], f16, isOutput=True
    )

    with tile.TileContext(nc) as tc:
        with (
            tc.tile_pool(name="const", bufs=1) as constp,
            tc.tile_pool(name="xp", bufs=2) as xp,
            tc.tile_pool(name="sqp", bufs=2) as sqp,
            tc.tile_pool(name="stp", bufs=2) as stp,
            tc.tile_pool(name="rowp", bufs=2) as rowp,
            tc.tile_pool(name="svtp", bufs=2) as svtp,
            tc.tile_pool(name="relup", bufs=2) as relup,
            tc.tile_pool(name="msump", bufs=2) as msump,
            tc.tile_pool(name="outp", bufs=2) as outp,
            tc.tile_pool(name="ps_main", bufs=2, space="PSUM") as ps_mainp,
            tc.tile_pool(name="ps_stat", bufs=2, space="PSUM") as ps_statp,
        ):
            wca_sb = constp.tile([C + 1, MO], f16)
            cst_sb = constp.tile([128, 8], f16)
            epsb = constp.tile([64, 1], f32)
            nc.sync.dma_start(out=wca_sb[:, :], in_=wca_d[:, :])
            nc.sync.dma_start(out=cst_sb[:, :], in_=cst_d[:, :])
            nc.gpsimd.memset(epsb[:, :], EPS)

            def emit_stats(ci):
                """Load x, compute sv (into xt row 64) and aT for chunk ci."""
                p0 = ci * chunk
                xt = xp.tile([C + 1, chunk], f16, tag="xt")
                nc.sync.dma_start(out=xt[0:C, :], in_=xin[:, p0 : p0 + chunk])

                sqt = sqp.tile([128, chunk], f16, tag="sqt")
                h = chunk // 4
                nc.vector.tensor_mul(
                    sqt[64:128, 0 : 3 * h], xt[0:C, 0 : 3 * h], xt[0:C, 0 : 3 * h]
                )
                nc.gpsimd.tensor_mul(
                    sqt[64:128, 3 * h : chunk],
                    xt[0:C, 3 * h : chunk],
                    xt[0:C, 3 * h : chunk],
                )

                st_x, st_s = [], []
                for hh in range(nslice // 8):
                    ps_x = ps_statp.tile([98, 512], f32, tag="ps_x")
                    ps_s = ps_statp.tile([98, 512], f32, tag="ps_s")
                    for k in range(4):
                        for r in range(2):
                            s = 8 * hh + 2 * k + r
                            sl = slice(512 * s, 512 * s + 512)
                            nc.tensor.matmul(
                                ps_x[32 * k : 32 * k + 2, :],
                                cst_sb[0:64, 2 * r : 2 * r + 2],
                                xt[0:C, sl],
                                start=(r == 0),
                                stop=(r == 1),
                                tile_position=(0, 32 * k),
                            )
                            nc.tensor.matmul(
                                ps_s[32 * k : 32 * k + 2, :],
                                cst_sb[64:128, 4 + 2 * r : 6 + 2 * r],
                                sqt[64:128, sl],
                                start=(r == 0),
                                stop=(r == 1),
                                tile_position=(64, 32 * k),
                            )
                    sx = stp.tile([128, 512], f32, tag=f"sx{hh}")
                    ss = stp.tile([128, 512], f32, tag=f"ss{hh}")
                    nc.scalar.activation(sx[0:98, :], ps_x[:, :], AF.Copy)
                    nc.scalar.activation(ss[0:98, :], ps_s[:, :], AF.Copy)
                    st_x.append(sx)
                    st_s.append(ss)

                muR = rowp.tile([64, 128], f32, tag="muR")
                e2R = rowp.tile([64, 128], f32, tag="e2R")
                for hh in range(nslice // 8):
                    for tiles, dstt in ((st_x, muR), (st_s, e2R)):
                        for k in range(4):
                            nc.sync.dma_start(
                                out=dstt[32 * hh + 8 * k : 32 * hh + 8 * k + 8, :],
                                in_=tiles[hh][32 * k : 32 * k + 2, :],
                            )

                musq = rowp.tile([64, 128], f32, tag="musq")
                varR = rowp.tile([64, 128], f32, tag="varR")
                svR = rowp.tile([64, 128], f32, tag="svR")
                nc.vector.tensor_mul(musq[:, :], muR[:, :], muR[:, :])
                nc.vector.tensor_sub(varR[:, :], e2R[:, :], musq[:, :])
                nc.scalar.activation(svR[:, :], varR[:, :], AF.Sqrt, bias=epsb[:, 0:1])
                svR16 = rowp.tile([64, 128], f16, tag="svR16")
                nc.vector.tensor_scalar_mul(svR16[:, :], svR[:, :], 1.0)
                nc.sync.dma_start(out=xt[C : C + 1, :], in_=svR16[:, :])

                aR32 = rowp.tile([64, 128], f32, tag="aR32")
                nc.vector.reciprocal_approx_fast(aR32[:, :], svR[:, :])
                aT32 = svtp.tile([128, 64], f32, tag="aT32")
                aT = svtp.tile([128, 64], f16, tag="aT")
                for a in range(4):
                    for b in range(2):
                        nc.vector.transpose(
                            aT32[32 * a : 32 * a + 32, 32 * b : 32 * b + 32],
                            aR32[32 * b : 32 * b + 32, 32 * a : 32 * a + 32],
                        )
                nc.vector.tensor_scalar_mul(aT[:, :], aT32[:, :], 1.0)
                return xt, aT

            def emit_mains(ci, xt, aT):
                """Main matmuls + drains + m-sum + scaled output for chunk ci."""
                p0 = ci * chunk
                osb = outp.tile([128, ntile * O], f16, tag="osb")
                for j in range(nspan):
                    ps = ps_mainp.tile([128, 1024], f32, tag="ps")
                    for i in range(4):
                        t = 4 * j + i
                        nc.tensor.matmul(
                            ps[:, 256 * i : 256 * (i + 1)],
                            xt[:, 128 * t : 128 * (t + 1)],
                            wca_sb[:, :],
                            start=True,
                            stop=True,
                        )
                    if j % 2 == 0:
                        rbuf = relup.tile([128, 2048], f16, tag="rbuf")
                    rsl = rbuf[:, 1024 * (j % 2) : 1024 * (j % 2) + 1024]
                    if (j * ACT_SPANS) // nspan != ((j + 1) * ACT_SPANS) // nspan:
                        nc.scalar.activation(rsl, ps[:, :], AF.Relu)
                    else:
                        nc.vector.tensor_scalar_max(rsl, ps[:, :], 0.0)
                    if j % 2 == 1:
                        g = j // 2
                        t1 = msump.tile([128, 1024], f16, tag="t1")
                        msum = msump.tile([128, 512], f16, tag="msum")
                        rv = rbuf[:, :].rearrange("p (t d) -> p t d", d=256)
                        t1v = t1[:, :].rearrange("p (t d) -> p t d", d=128)
                        nc.vector.tensor_add(t1v, rv[:, :, 0:128], rv[:, :, 128:256])
                        t1w = t1[:, :].rearrange("p (t d) -> p t d", d=128)
                        msv = msum[:, :].rearrange("p (t d) -> p t d", d=64)
                        nc.vector.tensor_add(msv, t1w[:, :, 0:64], t1w[:, :, 64:128])
                        ab = (
                            aT[:, 8 * g : 8 * g + 8]
                            .unsqueeze(2)
                            .to_broadcast((128, 8, 64))
                        )
                        ov = osb[:, 512 * g : 512 * g + 512].rearrange(
                            "p (t d) -> p t d", d=64
                        )
                        nc.vector.tensor_mul(ov, msv, ab)
                nc.sync.dma_start(
                    out=out_d[:, p0 // 128 * O : (p0 + chunk) // 128 * O],
                    in_=osb[:, :],
                )

            rep_ctx = tc.For_i(0, repeat, 1) if repeat > 1 else contextlib.nullcontext()
            with rep_ctx:
                # software pipeline: stats run one chunk ahead of mains so the
                # PE never stalls on the stats->sv serial chain
                prev = None
                for ci in range(nchunk + 1):
                    cur = emit_stats(ci) if ci < nchunk else None
                    if prev is not None:
                        emit_mains(ci - 1, *prev)
                    prev = cur

    nc.compile()
    return nc


def _host_consts(W, q, gamma, beta):
    W_eff = (
        W.astype(np.float64) * gamma.astype(np.float64)[None, None, :]
    ).reshape(MO, C)
    Wc = W_eff - W_eff.mean(axis=1, keepdims=True)
    bias = beta.astype(np.float64) @ W_eff.T  # [MO]
    bq = bias - np.float64(q)
    wca = np.concatenate([Wc.T, bq[None, :]], axis=0).astype(np.float16)  # [65,256]
    cst = np.zeros((128, 8), np.float16)
    cst[0:64, 0] = 1.0 / C  # mu weights, even slice (x-pass)
    cst[0:64, 3] = 1.0 / C  # mu weights, odd slice
    cst[64:128, 4] = 1.0 / C  # e2 weights, even slice (sq-pass)
    cst[64:128, 7] = 1.0 / C  # e2 weights, odd slice
    return wca, cst


def build_in_maps(inputs):
    x = np.ascontiguousarray(np.asarray(inputs["x"], dtype=np.float32))
    W = np.asarray(inputs["W"], dtype=np.float32)
    q = float(np.asarray(inputs["q"]).reshape(-1)[0])
    gamma = np.asarray(inputs["gamma"], dtype=np.float32)
    beta = np.asarray(inputs["beta"], dtype=np.float32)
    wca, cst = _host_consts(W, q, gamma, beta)
    xf = x.reshape(B, C, H * Wd)
    in_maps = []
    for k in range(NCORES):
        b, half = k // 2, k % 2
        xk = np.ascontiguousarray(
            xf[b, :, half * PIX_PER_CORE : (half + 1) * PIX_PER_CORE]
        ).astype(np.float16)
        in_maps.append({"xin": xk, "wca": wca, "cst": cst})
    return in_maps


def _run(inputs, trace=False):
    from concourse.bass_utils import run_bass_kernel_spmd

    in_maps = build_in_maps(inputs)
    if "nc" not in _cache:
        _cache["nc"] = _build()
    nc = _cache["nc"]

    res = run_bass_kernel_spmd(nc, in_maps, list(range(NCORES)), trace=trace)
    out = np.empty((B, O, H * Wd), np.float32)
    for k in range(NCORES):
        b, half = k // 2, k % 2
        od = res.results[k]["out"]  # [128, PIX/128*O] fp16
        # od[pp, 64*t + o] = out[px = 128*t + pp, o]
        tmp = od.reshape(128, PIX_PER_CORE // 128, O).astype(np.float32)
        tmp = tmp.transpose(2, 1, 0).reshape(O, PIX_PER_CORE)
        out[b, :, half * PIX_PER_CORE : (half + 1) * PIX_PER_CORE] = tmp
    return out.reshape(B, O, H, Wd), res.exec_time_ns


def kernel(**inputs) -> np.ndarray:
    out, _ = _run(inputs, trace=False)
    return out


if __name__ == "__main__":
    import reference

    inputs = reference.setup_inputs()
    expected = np.asarray(reference.reference(**inputs))
    got = kernel(**{k: np.asarray(v) for k, v in inputs.items()})
    diff = got.astype(np.float64) - expected.astype(np.float64)
    rel = np.linalg.norm(diff.ravel()) / np.linalg.norm(
        expected.astype(np.float64).ravel()
    )
    print("rel", rel, "maxabs", np.abs(diff).max())


# revision 8
# speedup vs baseline: 1.1994x; 1.1994x over previous
"""Trainium2 Bass kernel for nn_DNM_Conv_fold (LayerNorm + M parallel 1x1 convs
+ relu(y-q) summed over M), v2: fp16 end-to-end.

Math (same restructure as v1, validated):
  - gamma folds into W host-side; W rows centered => LayerNorm mean-subtraction
    implicit in the matmul.
  - sv = sqrt(var+eps) rides the matmul as contraction row 65 with weight row
    bq = beta@W_eff - q, so psum = y - q*sv and the drain is a PURE relu
    (batchable across tiles; no per-tile scalars).
  - out = a * sum_m relu(y - q*sv), a = 1/sv applied after the m-sum via a
    transposed per-tile scale table aT [128pp, 64tile].

Layout per core: x [64, 73728] fp16 + device-computed sv row -> xt [65, 8192]
per chunk. Main MM per 128-px tile: lhsT = xt[:, 128t:128t+128] (stationary),
rhs = wca [65, 256] (moving), psum [128px, 256mo]; 4 tiles per [128, 1024]
psum span; relu-drain spans on ACT or DVE (ratio tunable); m-sum via batched
fp16 DVE adds; scale via aT broadcast; output [128, 4096] fp16 pixel-major,
host unscrambles + upcasts.

Stats: 16 x 512-px slices/chunk; x-pass (rows 0-63) and sq-pass (rows 64-127
of a separate sq tile) matmuls write (mu, e2) rows into 32-aligned psum
col-groups, 4 slices/bank => 4 ACT bank-copies [98, 512] per chunk; DMA
gathers to [64, 128] row form; var/sqrt there; sv DMA'd into xt row 64; DVE
32x32 block-transposes give svT -> aT.

Sharding: 8 cores; core k = batch k//2, pixel half k%2 (73728 px each).
"""

import sys

sys.path.insert(0, "/opt/trn_rl_repo")

import numpy as np

# ---- problem constants (hardcoded; kernel.py must be self-contained) ----
B, C, O, M, H, Wd = 4, 64, 64, 4, 384, 384
EPS = 1e-5
MO = M * O  # 256
NCORES = 8
PIX_PER_CORE = B * H * Wd // NCORES  # 73728
CHUNK = 8192
NCHUNK = PIX_PER_CORE // CHUNK  # 9
NTILE = CHUNK // 128  # 64 px-tiles per chunk
NSPAN = NTILE // 4  # 16 drain spans (4 tiles each)
NSLICE = CHUNK // 512  # 16 stat slices
# drain-span engine schedule: True -> ACT, False -> DVE
ACT_SPANS = 16  # of 16 (all-ACT drains measured fastest: 260us vs 288-302us mixed)

_cache = {}


def _build(pix_per_core=PIX_PER_CORE, chunk=CHUNK, repeat=1):
    import contextlib

    from concourse import bacc, bass, tile

    mybir = bass.mybir
    f32 = mybir.dt.float32
    f16 = mybir.dt.float16
    AF = mybir.ActivationFunctionType
    ALU = mybir.AluOpType

    nchunk = pix_per_core // chunk
    ntile = chunk // 128
    nspan = ntile // 4
    nslice = chunk // 512

    nc = bacc.Bacc(None, target_bir_lowering=False)
    xin = nc.declare_dram_parameter("xin", [C, pix_per_core], f16, isOutput=False)
    wca_d = nc.declare_dram_parameter("wca", [C + 1, MO], f16, isOutput=False)
    cst_d = nc.declare_dram_parameter("cst", [128, 8], f16, isOutput=False)
    out_d = nc.declare_dram_parameter(
        "out", [128, pix_per_core // 128 * O], f16, isOutput=True
    )

    with tile.TileContext(nc) as tc:
        with (
            tc.tile_pool(name="const", bufs=1) as constp,
            tc.tile_pool(name="xp", bufs=2) as xp,
            tc.tile_pool(name="sqp", bufs=2) as sqp,
            tc.tile_pool(name="stp", bufs=2) as stp,
            tc.tile_pool(name="rowp", bufs=2) as rowp,
            tc.tile_pool(name="svtp", bufs=2) as svtp,
            tc.tile_pool(name="relup", bufs=2) as relup,
            tc.tile_pool(name="msump", bufs=2) as msump,
            tc.tile_pool(name="outp", bufs=2) as outp,
            tc.tile_pool(name="ps_main", bufs=2, space="PSUM") as ps_mainp,
            tc.tile_pool(name="ps_stat", bufs=2, space="PSUM") as ps_statp,
        ):
            wca_sb = constp.tile([C + 1, MO], f16)
            cst_sb = constp.tile([128, 8], f16)
            epsb = constp.tile([128, 1], f32)
            nc.sync.dma_start(out=wca_sb[:, :], in_=wca_d[:, :])
            nc.sync.dma_start(out=cst_sb[:, :], in_=cst_d[:, :])
            nc.gpsimd.memset(epsb[:, :], EPS)

            def emit_stats(ci):
                """Load x, compute sv (into xt row 64) and aT for chunk ci."""
                p0 = ci * chunk
                xt = xp.tile([C + 1, chunk], f16, tag="xt")
                nc.sync.dma_start(out=xt[0:C, :], in_=xin[:, p0 : p0 + chunk])

                sqt = sqp.tile([128, chunk], f16, tag="sqt")
                h = chunk // 4
                nc.vector.tensor_mul(
                    sqt[64:128, 0 : 3 * h], xt[0:C, 0 : 3 * h], xt[0:C, 0 : 3 * h]
                )
                nc.gpsimd.tensor_mul(
                    sqt[64:128, 3 * h : chunk],
                    xt[0:C, 3 * h : chunk],
                    xt[0:C, 3 * h : chunk],
                )

                st_x, st_s = [], []
                for hh in range(nslice // 8):
                    ps_x = ps_statp.tile([98, 512], f32, tag="ps_x")
                    ps_s = ps_statp.tile([98, 512], f32, tag="ps_s")
                    for k in range(4):
                        for r in range(2):
                            s = 8 * hh + 2 * k + r
                            sl = slice(512 * s, 512 * s + 512)
                            nc.tensor.matmul(
                                ps_x[32 * k : 32 * k + 2, :],
                                cst_sb[0:64, 2 * r : 2 * r + 2],
                                xt[0:C, sl],
                                start=(r == 0),
                                stop=(r == 1),
                                tile_position=(0, 32 * k),
                            )
                            nc.tensor.matmul(
                                ps_s[32 * k : 32 * k + 2, :],
                                cst_sb[64:128, 4 + 2 * r : 6 + 2 * r],
                                sqt[64:128, sl],
                                start=(r == 0),
                                stop=(r == 1),
                                tile_position=(64, 32 * k),
                            )
                    sx = stp.tile([128, 512], f32, tag=f"sx{hh}")
                    ss = stp.tile([128, 512], f32, tag=f"ss{hh}")
                    nc.scalar.activation(sx[0:98, :], ps_x[:, :], AF.Copy)
                    nc.scalar.activation(ss[0:98, :], ps_s[:, :], AF.Copy)
                    st_x.append(sx)
                    st_s.append(ss)

                muR = rowp.tile([ntile, 128], f32, tag="muR")
                e2R = rowp.tile([ntile, 128], f32, tag="e2R")
                for hh in range(nslice // 8):
                    for tiles, dstt in ((st_x, muR), (st_s, e2R)):
                        for k in range(4):
                            nc.sync.dma_start(
                                out=dstt[32 * hh + 8 * k : 32 * hh + 8 * k + 8, :],
                                in_=tiles[hh][32 * k : 32 * k + 2, :],
                            )

                musq = rowp.tile([ntile, 128], f32, tag="musq")
                varR = rowp.tile([ntile, 128], f32, tag="varR")
                svR = rowp.tile([ntile, 128], f32, tag="svR")
                nc.vector.tensor_mul(musq[:, :], muR[:, :], muR[:, :])
                nc.vector.tensor_sub(varR[:, :], e2R[:, :], musq[:, :])
                nc.scalar.activation(svR[:, :], varR[:, :], AF.Sqrt, bias=epsb[0:ntile, 0:1])
                svR16 = rowp.tile([ntile, 128], f16, tag="svR16")
                nc.vector.tensor_scalar_mul(svR16[:, :], svR[:, :], 1.0)
                nc.sync.dma_start(out=xt[C : C + 1, :], in_=svR16[:, :])

                aR32 = rowp.tile([ntile, 128], f32, tag="aR32")
                nc.vector.reciprocal_approx_fast(aR32[:, :], svR[:, :])
                aT32 = svtp.tile([128, ntile], f32, tag="aT32")
                aT = svtp.tile([128, ntile], f16, tag="aT")
                for a in range(4):
                    for b in range(ntile // 32):
                        nc.vector.transpose(
                            aT32[32 * a : 32 * a + 32, 32 * b : 32 * b + 32],
                            aR32[32 * b : 32 * b + 32, 32 * a : 32 * a + 32],
                        )
                nc.vector.tensor_scalar_mul(aT[:, :], aT32[:, :], 1.0)
                return xt, aT

            def emit_mains(ci, xt, aT):
                """Main matmuls + drains + m-sum + scaled output for chunk ci."""
                p0 = ci * chunk
                osb = outp.tile([128, ntile * O], f16, tag="osb")
                for j in range(nspan):
                    ps = ps_mainp.tile([128, 1024], f32, tag="ps")
                    for i in range(4):
                        t = 4 * j + i
                        nc.tensor.matmul(
                            ps[:, 256 * i : 256 * (i + 1)],
                            xt[:, 128 * t : 128 * (t + 1)],
                            wca_sb[:, :],
                            start=True,
                            stop=True,
                        )
                    if j % 2 == 0:
                        rbuf = relup.tile([128, 2048], f16, tag="rbuf")
                    rsl = rbuf[:, 1024 * (j % 2) : 1024 * (j % 2) + 1024]
                    if (j * ACT_SPANS) // nspan != ((j + 1) * ACT_SPANS) // nspan:
                        nc.scalar.activation(rsl, ps[:, :], AF.Relu)
                    else:
                        nc.vector.tensor_scalar_max(rsl, ps[:, :], 0.0)
                    if j % 2 == 1:
                        g = j // 2
                        t1 = msump.tile([128, 1024], f16, tag="t1")
                        msum = msump.tile([128, 512], f16, tag="msum")
                        rv = rbuf[:, :].rearrange("p (t d) -> p t d", d=256)
                        t1v = t1[:, :].rearrange("p (t d) -> p t d", d=128)
                        nc.vector.tensor_add(t1v, rv[:, :, 0:128], rv[:, :, 128:256])
                        t1w = t1[:, :].rearrange("p (t d) -> p t d", d=128)
                        msv = msum[:, :].rearrange("p (t d) -> p t d", d=64)
                        nc.vector.tensor_add(msv, t1w[:, :, 0:64], t1w[:, :, 64:128])
                        ab = (
                            aT[:, 8 * g : 8 * g + 8]
                            .unsqueeze(2)
                            .to_broadcast((128, 8, 64))
                        )
                        ov = osb[:, 512 * g : 512 * g + 512].rearrange(
                            "p (t d) -> p t d", d=64
                        )
                        nc.vector.tensor_mul(ov, msv, ab)
                nc.sync.dma_start(
                    out=out_d[:, p0 // 128 * O : (p0 + chunk) // 128 * O],
                    in_=osb[:, :],
                )

            rep_ctx = tc.For_i(0, repeat, 1) if repeat > 1 else contextlib.nullcontext()
            with rep_ctx:
                # software pipeline: stats run one chunk ahead of mains so the
                # PE never stalls on the stats->sv serial chain
                prev = None
                for ci in range(nchunk + 1):
                    cur = emit_stats(ci) if ci < nchunk else None
                    if prev is not None:
                        emit_mains(ci - 1, *prev)
                    prev = cur

    nc.compile()
    return nc


def _host_consts(W, q, gamma, beta):
    W_eff = (
        W.astype(np.float64) * gamma.astype(np.float64)[None, None, :]
    ).reshape(MO, C)
    Wc = W_eff - W_eff.mean(axis=1, keepdims=True)
    bias = beta.astype(np.float64) @ W_eff.T  # [MO]
    bq = bias - np.float64(q)
    wca = np.concatenate([Wc.T, bq[None, :]], axis=0).astype(np.float16)  # [65,256]
    cst = np.zeros((128, 8), np.float16)
    cst[0:64, 0] = 1.0 / C  # mu weights, even slice (x-pass)
    cst[0:64, 3] = 1.0 / C  # mu weights, odd slice
    cst[64:128, 4] = 1.0 / C  # e2 weights, even slice (sq-pass)
    cst[64:128, 7] = 1.0 / C  # e2 weights, odd slice
    return wca, cst


def build_in_maps(inputs):
    x = np.ascontiguousarray(np.asarray(inputs["x"], dtype=np.float32))
    W = np.asarray(inputs["W"], dtype=np.float32)
    q = float(np.asarray(inputs["q"]).reshape(-1)[0])
    gamma = np.asarray(inputs["gamma"], dtype=np.float32)
    beta = np.asarray(inputs["beta"], dtype=np.float32)
    wca, cst = _host_consts(W, q, gamma, beta)
    xf = x.reshape(B, C, H * Wd)
    in_maps = []
    for k in range(NCORES):
        b, half = k // 2, k % 2
        xk = np.ascontiguousarray(
            xf[b, :, half * PIX_PER_CORE : (half + 1) * PIX_PER_CORE]
        ).astype(np.float16)
        in_maps.append({"xin": xk, "wca": wca, "cst": cst})
    return in_maps


def _run(inputs, trace=False):
    from concourse.bass_utils import run_bass_kernel_spmd

    in_maps = build_in_maps(inputs)
    if "nc" not in _cache:
        _cache["nc"] = _build()
    nc = _cache["nc"]

    res = run_bass_kernel_spmd(nc, in_maps, list(range(NCORES)), trace=trace)
    out = np.empty((B, O, H * Wd), np.float32)
    for k in range(NCORES):
        b, half = k // 2, k % 2
        od = res.results[k]["out"]  # [128, PIX/128*O] fp16
        # od[pp, 64*t + o] = out[px = 128*t + pp, o]
        tmp = od.reshape(128, PIX_PER_CORE // 128, O).astype(np.float32)
        tmp = tmp.transpose(2, 1, 0).reshape(O, PIX_PER_CORE)
        out[b, :, half * PIX_PER_CORE : (half + 1) * PIX_PER_CORE] = tmp
    return out.reshape(B, O, H, Wd), res.exec_time_ns


def kernel(**inputs) -> np.ndarray:
    out, _ = _run(inputs, trace=False)
    return out


if __name__ == "__main__":
    import reference

    inputs = reference.setup_inputs()
    expected = np.asarray(reference.reference(**inputs))
    got = kernel(**{k: np.asarray(v) for k, v in inputs.items()})
    diff = got.astype(np.float64) - expected.astype(np.float64)
    rel = np.linalg.norm(diff.ravel()) / np.linalg.norm(
        expected.astype(np.float64).ravel()
    )
    print("rel", rel, "maxabs", np.abs(diff).max())
